# revision 4
# baseline (speedup 1.0000x reference)
"""MultiHeadAttention (B=4, N=2048, E=1024, H=16) on 8 TRN2 NeuronCores.

Sharding: core c handles batch b = c//2 and head-half hh = c%2 (8 heads,
512 embed dims). Each core computes Q/K/V projections for its 8 heads,
attention, and a partial output projection (contraction over its 512 c-dims).
Host sums the two partials per batch and adds the output bias.

All matmuls run as float32r (tf32 mantissa, fp32 accumulate) at full PE rate.
Layouts are chosen so no transposes are ever needed on device:
  - host ships x.T  [embed, tok] so projections contract embed on partitions
  - Q/K are produced transposed: QT/KT [dout, tok]
  - scores are computed directly as S.T [k, q] (contraction d<=64)
  - V is produced in natural [tok, dv] layout with a ones-column appended per
    head, so attn@V yields O.T [d, q] AND the softmax denominators in row 64
  - softmax skips max-subtraction (|scores/8| < ~3, exp is safe in fp32)
  - output projection consumes O.T directly; host transposes the result once

The attention phase is software-pipelined at head granularity so the PE
never idles long enough for the HAM clock gate to re-throttle it to
1.2 GHz: scores for head h+1 are interleaved (per 2-keytile chunk) with
attn@V for head h, the softmax-denominator broadcast matmul for head h is
deferred until the reciprocal has had time to complete, and the output
projection for q-block qb is spread one jt-group per pipeline slot across
q-block qb+1. PSUM budget (8 banks): score chunks 2x[128,2,512] (4) +
attn@V accum 2x[128,512] (2) + denom broadcast (1) + out-proj accum (1).
"""
import sys

sys.path.insert(0, "/opt/trn_rl_repo")

import numpy as np

B, N, E = 4, 2048, 1024
NCORES = 8
HH = 512          # embed dims (8 heads x 64) per core
D = 64
NHEAD = 8         # heads per core

_cache = {}


def _tf32(x):
    u = np.ascontiguousarray(x, dtype=np.float32).view(np.uint32)
    lsb = (u >> 13) & 1
    u = (u + 0x0FFF + lsb) & 0xFFFFE000
    return u.view(np.float32)


def _split_matmul_waits(nc, mybir):
    """fp32r self-loading matmuls cannot carry sync waits (walrus places
    them on the S3_LW struct which has no wait slot). Move every wait off
    Matmult instructions onto InstEventSemaphore instructions inserted
    just before, in block order."""
    n_fixed = 0
    for fn in nc.m.functions:
        for blk in fn.blocks:
            insts = blk.instructions
            i = 0
            while i < len(insts):
                inst = insts[i]
                si = inst.sync_info
                if inst.opcode == "Matmult" and si is not None and len(si.on_wait) > 0:
                    waits = list(si.on_wait)
                    si.on_wait = []
                    inst.sync_info = si
                    pos = i
                    for j in range(0, len(waits), 2):
                        ev = mybir.InstEventSemaphore(
                            name=f"mmgate_{inst.name}_{j}",
                            ins=[],
                            outs=[],
                            sync_info=mybir.SyncInfo(
                                on_wait=waits[j : j + 2], on_update=[]
                            ),
                        )
                        ev.engine = inst.engine
                        nc.register_instruction(ev)
                        insts.insert(pos, ev)
                        pos += 1
                        i += 1
                    n_fixed += 1
                i += 1
            blk.instructions = insts
    return n_fixed


def _build():
    import concourse.mybir as mybir
    import concourse.tile as tile
    import concourse.bacc as bacc

    F32 = mybir.dt.float32
    F32R = mybir.dt.float32r
    EXP = mybir.ActivationFunctionType.Exp

    nc = bacc.Bacc(trn_type="TRN2")

    xtq = nc.dram_tensor("xtq", [E, N], F32R, kind="ExternalInput")
    xtk = nc.dram_tensor("xtk", [E, N], F32R, kind="ExternalInput")
    xtv = nc.dram_tensor("xtv", [E, N], F32R, kind="ExternalInput")
    wqt = nc.dram_tensor("wqt", [E, HH], F32R, kind="ExternalInput")
    wkt = nc.dram_tensor("wkt", [E, HH], F32R, kind="ExternalInput")
    wvt = nc.dram_tensor("wvt", [E, HH], F32R, kind="ExternalInput")
    wot = nc.dram_tensor("wot", [HH, E], F32R, kind="ExternalInput")
    bq = nc.dram_tensor("bq", [HH], F32, kind="ExternalInput")
    bk = nc.dram_tensor("bk", [HH], F32, kind="ExternalInput")
    bv = nc.dram_tensor("bv", [HH], F32, kind="ExternalInput")
    po = nc.dram_tensor("po", [E, N], F32, kind="ExternalOutput")

    with tile.TileContext(nc) as tc:
        with (
            tc.tile_pool(name="consts", bufs=1) as consts,
            tc.tile_pool(name="qk", bufs=1) as qk_pool,
            tc.tile_pool(name="vx", bufs=1) as v_pool,
            tc.tile_pool(name="wo", bufs=1) as wo_pool,
        ):
            # ---------------- constants ----------------
            ones_f = consts.tile([1, 128], F32)
            nc.vector.memset(ones_f, 1.0)
            ones_r = consts.tile([1, 128], F32R)
            nc.vector.tensor_copy(ones_r, ones_f)
            onescol_f = consts.tile([128, NHEAD, 1], F32)
            nc.vector.memset(onescol_f, 1.0)

            bq_t = consts.tile([128, 4], F32)
            bk_t = consts.tile([128, 4], F32)
            nc.sync.dma_start(out=bq_t, in_=bq.ap().rearrange("(t p) -> p t", p=128))
            nc.sync.dma_start(out=bk_t, in_=bk.ap().rearrange("(t p) -> p t", p=128))
            bv_row = consts.tile([1, HH], F32)
            nc.sync.dma_start(out=bv_row, in_=bv.ap().rearrange("(a n) -> a n", a=1))
            bv_row_r = consts.tile([1, HH], F32R)
            nc.vector.tensor_copy(bv_row_r, bv_row)
            bv_bc = consts.tile([128, HH], F32)

            # persistent activations
            QT = [qk_pool.tile([128, N], F32R, tag=f"qt{t}", name=f"qt{t}") for t in range(4)]
            KT = [qk_pool.tile([128, N], F32R, tag=f"kt{t}", name=f"kt{t}") for t in range(4)]
            VE = [v_pool.tile([128, NHEAD, D + 1], F32R, tag=f"ve{g}", name=f"ve{g}") for g in range(16)]
            wo_t = wo_pool.tile([128, 4, E], F32R, tag="wo")

            # ---------------- projections ----------------
            with (
                tc.tile_pool(name="w", bufs=2) as w_pool,
                tc.tile_pool(name="xt", bufs=2) as xt_pool,
                tc.tile_pool(name="pps", bufs=4, space="PSUM") as proj_ps,
            ):
                # broadcast bv to all partitions via K=1 matmul
                bc0 = proj_ps.tile([128, HH], F32, tag="bvbc")
                nc.tensor.matmul(bc0, ones_r, bv_row_r, start=True, stop=True)
                nc.vector.tensor_copy(bv_bc, bc0)

                w_tiles = {}
                for name, wdram in (("q", wqt), ("k", wkt), ("v", wvt)):
                    wt = w_pool.tile([128, 8, HH], F32R, tag="w")
                    nc.sync.dma_start(
                        out=wt, in_=wdram.ap().rearrange("(kt p) n -> p kt n", p=128)
                    )
                    w_tiles[name] = wt

                def qk_proj(xdram, wt, dest, bias_t):
                    for th in range(4):
                        xt = xt_pool.tile([128, 8, 512], F32R, tag="xt")
                        nc.sync.dma_start(
                            out=xt,
                            in_=xdram.ap().rearrange("(kt p) n -> p kt n", p=128)[
                                :, :, 512 * th : 512 * (th + 1)
                            ],
                        )
                        for dt_ in range(4):
                            ps = proj_ps.tile([128, 512], F32, tag="pp")
                            for kt in range(8):
                                nc.tensor.matmul(
                                    ps,
                                    wt[:, kt, 128 * dt_ : 128 * (dt_ + 1)],
                                    xt[:, kt, :],
                                    start=(kt == 0),
                                    stop=(kt == 7),
                                )
                            off = 512 * th
                            nc.vector.tensor_scalar_add(
                                dest[dt_][:, off : off + 512],
                                ps,
                                bias_t[:, dt_ : dt_ + 1],
                            )

                qk_proj(xtq, w_tiles["q"], QT, bq_t)
                qk_proj(xtk, w_tiles["k"], KT, bk_t)

                # V in natural [tok, dv] layout + ones column
                for th in range(4):
                    xt = xt_pool.tile([128, 8, 512], F32R, tag="xt")
                    nc.sync.dma_start(
                        out=xt,
                        in_=xtv.ap().rearrange("(kt p) n -> p kt n", p=128)[
                            :, :, 512 * th : 512 * (th + 1)
                        ],
                    )
                    for tt in range(4):
                        g = 4 * th + tt
                        ps = proj_ps.tile([128, 512], F32, tag="pp")
                        for kt in range(8):
                            nc.tensor.matmul(
                                ps,
                                xt[:, kt, 128 * tt : 128 * (tt + 1)],
                                w_tiles["v"][:, kt, :],
                                start=(kt == 0),
                                stop=(kt == 7),
                            )
                        nc.vector.tensor_add(
                            VE[g][:, :, 0:D],
                            ps.rearrange("p (h d) -> p h d", h=NHEAD),
                            bv_bc.rearrange("p (h d) -> p h d", h=NHEAD),
                        )
                        nc.vector.tensor_copy(VE[g][:, :, D : D + 1], onescol_f)

                # output projection weights (loaded during attention DMA slack)
                nc.sync.dma_start(
                    out=wo_t, in_=wot.ap().rearrange("(ct p) n -> p ct n", p=128)
                )

            # ---------------- attention (head-pipelined) ----------------
            # Linear head index L = qb*8 + h. Pipeline slot s: issue score
            # chunks for L_S = s and attn@V chunks for L_V = s-1, chunk c
            # covering key tiles 2c, 2c+1. Per chunk the PE does 2 score MMs
            # + 2 attn@V MMs (~850ns warm) while the ACT does one exp over
            # [128,2,512] (~1.05us) — ACT paces, PE never starves.
            with (
                tc.tile_pool(name="attn", bufs=12) as at_pool,
                tc.tile_pool(name="otn", bufs=2) as otn_pool,
                tc.tile_pool(name="small", bufs=2) as small_pool,
                tc.tile_pool(name="ostage", bufs=2) as ostage_pool,
                tc.tile_pool(name="st_ps", bufs=2, space="PSUM") as st_ps,
                tc.tile_pool(name="ot_ps", bufs=2, space="PSUM") as ot_ps,
                tc.tile_pool(name="bc_ps", bufs=1, space="PSUM") as bc_ps,
                tc.tile_pool(name="oj_ps", bufs=1, space="PSUM") as oj_ps,
            ):
                NL = 4 * NHEAD          # 32 (qb, h) pairs
                at_tiles = {}           # (L, c) -> SBUF exp tile
                ot_tiles = {}           # L -> PSUM attn@V accumulator
                rs = {}                 # L -> reciprocal of denominators
                otn_by_qb = {}          # qb -> 4 otn SBUF tiles
                pending_oj = []

                def issue_scores(L, c):
                    qb, h = divmod(L, NHEAD)
                    t, par = h // 2, (h % 2) * 64
                    q0 = 512 * qb
                    st = st_ps.tile([128, 2, 512], F32, tag="st", name=f"st{L}_{c}")
                    for i in range(2):
                        kt = 2 * c + i
                        nc.tensor.matmul(
                            st[:, i, :],
                            KT[t][par : par + 64, 128 * kt : 128 * (kt + 1)],
                            QT[t][par : par + 64, q0 : q0 + 512],
                            start=True,
                            stop=True,
                        )
                    at = at_pool.tile([128, 2, 512], F32R, tag="at", name=f"at{L}_{c}")
                    nc.scalar.activation(at, st, EXP, scale=0.125)
                    at_tiles[(L, c)] = at

                def issue_attnv(L, c):
                    h = L % NHEAD
                    if c == 0:
                        ot_tiles[L] = ot_ps.tile(
                            [128, 512], F32, tag="ot", name=f"ot{L}"
                        )
                    ot = ot_tiles[L]
                    at = at_tiles.pop((L, c))
                    for i in range(2):
                        kt = 2 * c + i
                        nc.tensor.matmul(
                            ot[0:65, :],
                            VE[kt][:, h, :],
                            at[:, i, :],
                            start=(kt == 0),
                            stop=(kt == 15),
                        )

                def issue_recip(L):
                    ot = ot_tiles[L]
                    r = small_pool.tile([1, 512], F32, tag="r", name=f"r{L}")
                    nc.vector.reciprocal_approx_fast(r, ot[64:65, :])
                    r_r = small_pool.tile([1, 512], F32R, tag="r_r", name=f"rr{L}")
                    nc.vector.tensor_copy(r_r, r)  # rounds to tf32 for the bc matmul
                    rs[L] = r_r

                def issue_bcmul(L):
                    qb, h = divmod(L, NHEAD)
                    t, par = h // 2, (h % 2) * 64
                    ot = ot_tiles.pop(L)
                    r = rs.pop(L)
                    bc = bc_ps.tile([128, 512], F32, tag="bc", name=f"bc{L}")
                    nc.tensor.matmul(
                        bc[0:64, :], ones_r[:, 0:64], r, start=True, stop=True
                    )
                    rbc = small_pool.tile([64, 512], F32, tag="rbc", name=f"rbc{L}")
                    nc.vector.tensor_copy(rbc, bc[0:64, :])
                    nc.vector.tensor_mul(
                        otn_by_qb[qb][t][par : par + 64, :], ot[0:64, :], rbc
                    )

                def issue_oj(qb, jt):
                    pj = oj_ps.tile([128, 512], F32, tag="oj", name=f"oj{qb}_{jt}")
                    for ct in range(4):
                        nc.tensor.matmul(
                            pj,
                            wo_t[:, ct, 128 * jt : 128 * (jt + 1)],
                            otn_by_qb[qb][ct],
                            start=(ct == 0),
                            stop=(ct == 3),
                        )
                    oj_sb = ostage_pool.tile(
                        [128, 512], F32, tag="oj_sb", name=f"ojsb{qb}_{jt}"
                    )
                    nc.vector.tensor_copy(oj_sb, pj)
                    nc.sync.dma_start(
                        out=po.ap()[128 * jt : 128 * (jt + 1), 512 * qb : 512 * (qb + 1)],
                        in_=oj_sb,
                    )
                    if jt == 7:
                        del otn_by_qb[qb]

                for s in range(NL + 1):
                    L_S = s if s < NL else None
                    L_V = s - 1 if s >= 1 else None
                    if L_S is not None and L_S % NHEAD == 0:
                        qb_S = L_S // NHEAD
                        otn_by_qb[qb_S] = [
                            otn_pool.tile(
                                [128, 512], F32R, tag=f"otn{ct}",
                                name=f"otn{ct}_{qb_S}",
                            )
                            for ct in range(4)
                        ]
                    if L_S is not None and L_S % NHEAD == 2 and L_S // NHEAD >= 1:
                        pending_oj += [(L_S // NHEAD - 1, jt) for jt in range(8)]
                    for c in range(8):
                        if L_V is not None:
                            issue_attnv(L_V, c)
                        if c == 2 and L_V is not None and L_V >= 1:
                            issue_bcmul(L_V - 1)
                        if c == 5 and pending_oj:
                            issue_oj(*pending_oj.pop(0))
                        if L_S is not None:
                            issue_scores(L_S, c)
                    if L_V is not None:
                        issue_recip(L_V)

                # tail: normalization of the last head + remaining out-proj
                issue_bcmul(NL - 1)
                while pending_oj:
                    issue_oj(*pending_oj.pop(0))
                for jt in range(8):
                    issue_oj(3, jt)

    nc.compile()
    _split_matmul_waits(nc, mybir)
    return nc


def _get_nc():
    if "nc" not in _cache:
        _cache["nc"] = _build()
    return _cache["nc"]


def kernel(query, key, value, Wq, bq, Wk, bk, Wv, bv, Wo, bo):
    from concourse.bass_utils import run_bass_kernel_spmd

    nc = _get_nc()

    query = np.asarray(query, dtype=np.float32)
    key = np.asarray(key, dtype=np.float32)
    value = np.asarray(value, dtype=np.float32)
    Wq, Wk, Wv, Wo = (np.asarray(w, dtype=np.float32) for w in (Wq, Wk, Wv, Wo))
    bq, bk, bv, bo = (np.asarray(b, dtype=np.float32) for b in (bq, bk, bv, bo))

    in_maps = []
    for c in range(NCORES):
        b, hh = c // 2, c % 2
        cols = slice(HH * hh, HH * (hh + 1))
        in_maps.append(
            {
                "xtq": _tf32(query[b].T),
                "xtk": _tf32(key[b].T),
                "xtv": _tf32(value[b].T),
                "wqt": _tf32(Wq[cols, :].T),
                "wkt": _tf32(Wk[cols, :].T),
                "wvt": _tf32(Wv[cols, :].T),
                "wot": _tf32(Wo[:, cols].T),
                "bq": bq[cols],
                "bk": bk[cols],
                "bv": bv[cols],
            }
        )

    _cache["in_maps"] = in_maps
    res = run_bass_kernel_spmd(nc, in_maps, core_ids=list(range(NCORES)))
    out = np.empty((B, N, E), dtype=np.float32)
    for b in range(B):
        p = res.results[2 * b]["po"] + res.results[2 * b + 1]["po"]
        out[b] = p.T + bo
    return out


# revision 7
# speedup vs baseline: 1.1299x; 1.1299x over previous
"""MultiHeadAttention (B=4, N=2048, E=1024, H=16) on 8 TRN2 NeuronCores.

Sharding: core c handles batch b = c//2 and head-half hh = c%2 (8 heads,
512 embed dims). Each core computes Q/K/V projections for its 8 heads,
attention, and a partial output projection (contraction over its 512 c-dims).
Host sums the two partials per batch and adds the output bias.

All matmuls run as float32r (tf32 mantissa, fp32 accumulate) at full PE rate.
Layouts are chosen so no transposes are ever needed on device:
  - host ships x.T  [embed, tok] so projections contract embed on partitions
  - Q/K are produced transposed: QT/KT [dout, tok]
  - scores are computed directly as S.T [k, q] (contraction d<=64)
  - V is produced in natural [tok, dv] layout with a ones-column appended per
    head, so attn@V yields O.T [d, q] AND the softmax denominators in row 64
  - softmax skips max-subtraction (|scores/8| < ~3, exp is safe in fp32)
  - output projection consumes O.T directly; host transposes the result once

The attention phase is software-pipelined at head granularity so the PE
never idles long enough for the HAM clock gate to re-throttle it to
1.2 GHz: scores for head h+1 are interleaved (per 2-keytile chunk) with
attn@V for head h, the softmax-denominator broadcast matmul for head h is
deferred until the reciprocal has had time to complete, and the output
projection for q-block qb is spread one jt-group per pipeline slot across
q-block qb+1. PSUM budget (8 banks): score chunks 2x[128,2,512] (4) +
attn@V accum 2x[128,512] (2) + denom broadcast (1) + out-proj accum (1).
"""
import sys

sys.path.insert(0, "/opt/trn_rl_repo")

import numpy as np
import ml_dtypes

B, N, E = 4, 2048, 1024
NCORES = 8
HH = 512          # embed dims (8 heads x 64) per core
D = 64
NHEAD = 8         # heads per core

_cache = {}


def _tf32(x):
    u = np.ascontiguousarray(x, dtype=np.float32).view(np.uint32)
    lsb = (u >> 13) & 1
    u = (u + 0x0FFF + lsb) & 0xFFFFE000
    return u.view(np.float32)


def _split_matmul_waits(nc, mybir):
    """fp32r self-loading matmuls cannot carry sync waits (walrus places
    them on the S3_LW struct which has no wait slot). Move every wait off
    fp32r Matmult instructions onto InstEventSemaphore instructions
    inserted just before, in block order.

    Non-fp32r matmuls MUST keep their native waits: walrus lowers them to
    LDWEIGHTS+MATMUL with the wait on the LDWEIGHTS, and the PE's silicon
    reorder window can hoist a wait-less LDWEIGHTS past a preceding
    EventSemaphore — reading the stationary operand before its producer
    wrote it."""

    def _is_f32r(inst):
        try:
            return any(
                str(getattr(ap, "dtype", "")) in ("float32r", "dt.float32r")
                or getattr(ap, "dtype", None) == mybir.dt.float32r
                for ap in inst.ins
            )
        except Exception:
            return True

    n_fixed = 0
    for fn in nc.m.functions:
        for blk in fn.blocks:
            insts = blk.instructions
            i = 0
            while i < len(insts):
                inst = insts[i]
                si = inst.sync_info
                if (
                    inst.opcode == "Matmult"
                    and si is not None
                    and len(si.on_wait) > 0
                    and _is_f32r(inst)
                ):
                    waits = list(si.on_wait)
                    si.on_wait = []
                    inst.sync_info = si
                    pos = i
                    for j in range(0, len(waits), 2):
                        ev = mybir.InstEventSemaphore(
                            name=f"mmgate_{inst.name}_{j}",
                            ins=[],
                            outs=[],
                            sync_info=mybir.SyncInfo(
                                on_wait=waits[j : j + 2], on_update=[]
                            ),
                        )
                        ev.engine = inst.engine
                        nc.register_instruction(ev)
                        insts.insert(pos, ev)
                        pos += 1
                        i += 1
                    n_fixed += 1
                i += 1
            blk.instructions = insts
    return n_fixed


def _build():
    import concourse.mybir as mybir
    import concourse.tile as tile
    import concourse.bacc as bacc

    F32 = mybir.dt.float32
    F32R = mybir.dt.float32r
    BF16 = mybir.dt.bfloat16
    EXP = mybir.ActivationFunctionType.Exp

    nc = bacc.Bacc(trn_type="TRN2")

    xtq = nc.dram_tensor("xtq", [E, N], F32R, kind="ExternalInput")
    xtk = nc.dram_tensor("xtk", [E, N], F32R, kind="ExternalInput")
    xtv = nc.dram_tensor("xtv", [E, N], F32R, kind="ExternalInput")
    wqt = nc.dram_tensor("wqt", [E, HH], F32R, kind="ExternalInput")
    wkt = nc.dram_tensor("wkt", [E, HH], F32R, kind="ExternalInput")
    wvt = nc.dram_tensor("wvt", [E, HH], F32R, kind="ExternalInput")
    wot = nc.dram_tensor("wot", [HH, E], BF16, kind="ExternalInput")
    bq = nc.dram_tensor("bq", [HH], F32, kind="ExternalInput")
    bk = nc.dram_tensor("bk", [HH], F32, kind="ExternalInput")
    bv = nc.dram_tensor("bv", [HH], F32, kind="ExternalInput")
    po = nc.dram_tensor("po", [E, N], F32, kind="ExternalOutput")

    with tile.TileContext(nc) as tc:
        with (
            tc.tile_pool(name="consts", bufs=1) as consts,
            tc.tile_pool(name="qk", bufs=1) as qk_pool,
            tc.tile_pool(name="vx", bufs=1) as v_pool,
            tc.tile_pool(name="wo", bufs=1) as wo_pool,
        ):
            # ---------------- constants ----------------
            ones_f = consts.tile([1, 128], F32)
            nc.vector.memset(ones_f, 1.0)
            ones_r = consts.tile([1, 128], F32R)
            nc.vector.tensor_copy(ones_r, ones_f)
            onescol_f = consts.tile([128, NHEAD, 1], F32)
            nc.vector.memset(onescol_f, 1.0)

            bq_t = consts.tile([128, 4], F32)
            bk_t = consts.tile([128, 4], F32)
            nc.sync.dma_start(out=bq_t, in_=bq.ap().rearrange("(t p) -> p t", p=128))
            nc.sync.dma_start(out=bk_t, in_=bk.ap().rearrange("(t p) -> p t", p=128))
            bv_row = consts.tile([1, HH], F32)
            nc.sync.dma_start(out=bv_row, in_=bv.ap().rearrange("(a n) -> a n", a=1))
            bv_row_r = consts.tile([1, HH], F32R)
            nc.vector.tensor_copy(bv_row_r, bv_row)
            bv_bc = consts.tile([128, HH], F32)

            # persistent activations
            QT = [qk_pool.tile([128, N], BF16, tag=f"qt{t}", name=f"qt{t}") for t in range(4)]
            KT = [qk_pool.tile([128, N], BF16, tag=f"kt{t}", name=f"kt{t}") for t in range(4)]
            VE = [v_pool.tile([128, NHEAD, D + 1], BF16, tag=f"ve{g}", name=f"ve{g}") for g in range(16)]
            wo_t = wo_pool.tile([128, 4, E], BF16, tag="wo")

            # ---------------- projections ----------------
            with (
                tc.tile_pool(name="w", bufs=2) as w_pool,
                tc.tile_pool(name="xt", bufs=2) as xt_pool,
                tc.tile_pool(name="pps", bufs=4, space="PSUM") as proj_ps,
            ):
                # broadcast bv to all partitions via K=1 matmul
                bc0 = proj_ps.tile([128, HH], F32, tag="bvbc")
                nc.tensor.matmul(bc0, ones_r, bv_row_r, start=True, stop=True)
                nc.vector.tensor_copy(bv_bc, bc0)

                w_tiles = {}
                for name, wdram in (("q", wqt), ("k", wkt), ("v", wvt)):
                    wt = w_pool.tile([128, 8, HH], F32R, tag="w")
                    nc.sync.dma_start(
                        out=wt, in_=wdram.ap().rearrange("(kt p) n -> p kt n", p=128)
                    )
                    w_tiles[name] = wt

                def qk_proj(xdram, wt, dest, bias_t):
                    for th in range(4):
                        xt = xt_pool.tile([128, 8, 512], F32R, tag="xt")
                        nc.sync.dma_start(
                            out=xt,
                            in_=xdram.ap().rearrange("(kt p) n -> p kt n", p=128)[
                                :, :, 512 * th : 512 * (th + 1)
                            ],
                        )
                        for dt_ in range(4):
                            ps = proj_ps.tile([128, 512], F32, tag="pp")
                            for kt in range(8):
                                nc.tensor.matmul(
                                    ps,
                                    wt[:, kt, 128 * dt_ : 128 * (dt_ + 1)],
                                    xt[:, kt, :],
                                    start=(kt == 0),
                                    stop=(kt == 7),
                                )
                            off = 512 * th
                            nc.vector.tensor_scalar_add(
                                dest[dt_][:, off : off + 512],
                                ps,
                                bias_t[:, dt_ : dt_ + 1],
                            )

                qk_proj(xtq, w_tiles["q"], QT, bq_t)
                qk_proj(xtk, w_tiles["k"], KT, bk_t)

                # V in natural [tok, dv] layout + ones column
                for th in range(4):
                    xt = xt_pool.tile([128, 8, 512], F32R, tag="xt")
                    nc.sync.dma_start(
                        out=xt,
                        in_=xtv.ap().rearrange("(kt p) n -> p kt n", p=128)[
                            :, :, 512 * th : 512 * (th + 1)
                        ],
                    )
                    for tt in range(4):
                        g = 4 * th + tt
                        ps = proj_ps.tile([128, 512], F32, tag="pp")
                        for kt in range(8):
                            nc.tensor.matmul(
                                ps,
                                xt[:, kt, 128 * tt : 128 * (tt + 1)],
                                w_tiles["v"][:, kt, :],
                                start=(kt == 0),
                                stop=(kt == 7),
                            )
                        nc.vector.tensor_add(
                            VE[g][:, :, 0:D],
                            ps.rearrange("p (h d) -> p h d", h=NHEAD),
                            bv_bc.rearrange("p (h d) -> p h d", h=NHEAD),
                        )
                        nc.vector.tensor_copy(VE[g][:, :, D : D + 1], onescol_f)

                # output projection weights (loaded during attention DMA slack)
                nc.sync.dma_start(
                    out=wo_t, in_=wot.ap().rearrange("(ct p) n -> p ct n", p=128)
                )

            # ---------------- attention (head-pipelined) ----------------
            # Linear head index L = qb*8 + h. Pipeline slot s: issue score
            # chunks for L_S = s and attn@V chunks for L_V = s-1, chunk c
            # covering key tiles 2c, 2c+1. Per chunk the PE does 2 score MMs
            # + 2 attn@V MMs (~850ns warm) while the ACT does one exp over
            # [128,2,512] (~1.05us) — ACT paces, PE never starves.
            with (
                tc.tile_pool(name="attn", bufs=12) as at_pool,
                tc.tile_pool(name="otn", bufs=2) as otn_pool,
                tc.tile_pool(name="small", bufs=2) as small_pool,
                tc.tile_pool(name="ostage", bufs=2) as ostage_pool,
                tc.tile_pool(name="st_ps", bufs=2, space="PSUM") as st_ps,
                tc.tile_pool(name="ot_ps", bufs=2, space="PSUM") as ot_ps,
                tc.tile_pool(name="bc_ps", bufs=1, space="PSUM") as bc_ps,
                tc.tile_pool(name="oj_ps", bufs=1, space="PSUM") as oj_ps,
            ):
                NL = 4 * NHEAD          # 32 (qb, h) pairs
                at_tiles = {}           # (L, c) -> SBUF exp tile
                ot_tiles = {}           # L -> PSUM attn@V accumulator
                rs = {}                 # L -> reciprocal of denominators
                otn_by_qb = {}          # qb -> 4 otn SBUF tiles
                pending_oj = []

                def issue_scores(L, c):
                    qb, h = divmod(L, NHEAD)
                    t, par = h // 2, (h % 2) * 64
                    q0 = 512 * qb
                    st = st_ps.tile([128, 2, 512], F32, tag="st", name=f"st{L}_{c}")
                    for i in range(2):
                        kt = 2 * c + i
                        nc.tensor.matmul(
                            st[:, i, :],
                            KT[t][par : par + 64, 128 * kt : 128 * (kt + 1)],
                            QT[t][par : par + 64, q0 : q0 + 512],
                            start=True,
                            stop=True,
                        )
                    at = at_pool.tile([128, 2, 512], BF16, tag="at", name=f"at{L}_{c}")
                    nc.scalar.activation(at, st, EXP, scale=0.125)
                    at_tiles[(L, c)] = at

                def issue_attnv(L, c):
                    h = L % NHEAD
                    if c == 0:
                        ot_tiles[L] = ot_ps.tile(
                            [128, 512], F32, tag="ot", name=f"ot{L}"
                        )
                    ot = ot_tiles[L]
                    at = at_tiles.pop((L, c))
                    for i in range(2):
                        kt = 2 * c + i
                        nc.tensor.matmul(
                            ot[0:65, :],
                            VE[kt][:, h, :],
                            at[:, i, :],
                            start=(kt == 0),
                            stop=(kt == 15),
                        )

                def issue_recip(L):
                    ot = ot_tiles[L]
                    r = small_pool.tile([1, 512], F32R, tag="r", name=f"r{L}")
                    with nc.allow_low_precision(reason="tf32 softmax denom"):
                        nc.vector.reciprocal(r, ot[64:65, :])
                    rs[L] = r

                def issue_bcmul(L):
                    qb, h = divmod(L, NHEAD)
                    t, par = h // 2, (h % 2) * 64
                    ot = ot_tiles.pop(L)
                    r = rs.pop(L)
                    bc = bc_ps.tile([128, 512], F32, tag="bc", name=f"bc{L}")
                    nc.tensor.matmul(
                        bc[0:64, :], ones_r[:, 0:64], r, start=True, stop=True
                    )
                    rbc = small_pool.tile([64, 512], F32, tag="rbc", name=f"rbc{L}")
                    nc.vector.tensor_copy(rbc, bc[0:64, :])
                    nc.vector.tensor_mul(
                        otn_by_qb[qb][t][par : par + 64, :], ot[0:64, :], rbc
                    )

                def issue_oj(qb, jt):
                    pj = oj_ps.tile([128, 512], F32, tag="oj", name=f"oj{qb}_{jt}")
                    for ct in range(4):
                        nc.tensor.matmul(
                            pj,
                            wo_t[:, ct, 128 * jt : 128 * (jt + 1)],
                            otn_by_qb[qb][ct],
                            start=(ct == 0),
                            stop=(ct == 3),
                        )
                    oj_sb = ostage_pool.tile(
                        [128, 512], F32, tag="oj_sb", name=f"ojsb{qb}_{jt}"
                    )
                    nc.vector.tensor_copy(oj_sb, pj)
                    nc.sync.dma_start(
                        out=po.ap()[128 * jt : 128 * (jt + 1), 512 * qb : 512 * (qb + 1)],
                        in_=oj_sb,
                    )
                    if jt == 7:
                        del otn_by_qb[qb]

                for s in range(NL + 1):
                    L_S = s if s < NL else None
                    L_V = s - 1 if s >= 1 else None
                    if L_S is not None and L_S % NHEAD == 0:
                        qb_S = L_S // NHEAD
                        otn_by_qb[qb_S] = [
                            otn_pool.tile(
                                [128, 512], BF16, tag=f"otn{ct}",
                                name=f"otn{ct}_{qb_S}",
                            )
                            for ct in range(4)
                        ]
                    if L_S is not None and L_S % NHEAD == 2 and L_S // NHEAD >= 1:
                        pending_oj += [(L_S // NHEAD - 1, jt) for jt in range(8)]
                    for c in range(8):
                        if L_V is not None:
                            issue_attnv(L_V, c)
                        if c == 2 and L_V is not None and L_V >= 1:
                            issue_bcmul(L_V - 1)
                        if c == 5 and pending_oj:
                            issue_oj(*pending_oj.pop(0))
                        if L_S is not None:
                            issue_scores(L_S, c)
                    if L_V is not None:
                        issue_recip(L_V)

                # tail: normalization of the last head + remaining out-proj
                issue_bcmul(NL - 1)
                while pending_oj:
                    issue_oj(*pending_oj.pop(0))
                for jt in range(8):
                    issue_oj(3, jt)

    nc.compile()
    _split_matmul_waits(nc, mybir)
    return nc


def _get_nc():
    if "nc" not in _cache:
        _cache["nc"] = _build()
    return _cache["nc"]


def kernel(query, key, value, Wq, bq, Wk, bk, Wv, bv, Wo, bo):
    from concourse.bass_utils import run_bass_kernel_spmd

    nc = _get_nc()

    query = np.asarray(query, dtype=np.float32)
    key = np.asarray(key, dtype=np.float32)
    value = np.asarray(value, dtype=np.float32)
    Wq, Wk, Wv, Wo = (np.asarray(w, dtype=np.float32) for w in (Wq, Wk, Wv, Wo))
    bq, bk, bv, bo = (np.asarray(b, dtype=np.float32) for b in (bq, bk, bv, bo))

    in_maps = []
    for c in range(NCORES):
        b, hh = c // 2, c % 2
        cols = slice(HH * hh, HH * (hh + 1))
        in_maps.append(
            {
                "xtq": _tf32(query[b].T),
                "xtk": _tf32(key[b].T),
                "xtv": _tf32(value[b].T),
                "wqt": _tf32(Wq[cols, :].T),
                "wkt": _tf32(Wk[cols, :].T),
                "wvt": _tf32(Wv[cols, :].T),
                "wot": np.ascontiguousarray(Wo[:, cols].T).astype(ml_dtypes.bfloat16),
                "bq": bq[cols],
                "bk": bk[cols],
                "bv": bv[cols],
            }
        )

    _cache["in_maps"] = in_maps
    res = run_bass_kernel_spmd(nc, in_maps, core_ids=list(range(NCORES)))
    out = np.empty((B, N, E), dtype=np.float32)
    for b in range(B):
        p = res.results[2 * b]["po"] + res.results[2 * b + 1]["po"]
        out[b] = p.T + bo
    return out


# revision 8
# speedup vs baseline: 1.3207x; 1.1689x over previous
"""MultiHeadAttention (B=4, N=2048, E=1024, H=16) on 8 TRN2 NeuronCores.

Sharding: core c handles batch b = c//2 and head-half hh = c%2 (8 heads,
512 embed dims). Each core computes Q/K/V projections for its 8 heads,
attention, and a partial output projection (contraction over its 512 c-dims).
Host sums the two partials per batch and adds the output bias.

All matmuls run as float32r (tf32 mantissa, fp32 accumulate) at full PE rate.
Layouts are chosen so no transposes are ever needed on device:
  - host ships x.T  [embed, tok] so projections contract embed on partitions
  - Q/K are produced transposed: QT/KT [dout, tok]
  - scores are computed directly as S.T [k, q] (contraction d<=64)
  - V is produced in natural [tok, dv] layout with a ones-column appended per
    head, so attn@V yields O.T [d, q] AND the softmax denominators in row 64
  - softmax skips max-subtraction (|scores/8| < ~3, exp is safe in fp32)
  - output projection consumes O.T directly; host transposes the result once

The attention phase is software-pipelined at head granularity so the PE
never idles long enough for the HAM clock gate to re-throttle it to
1.2 GHz: scores for head h+1 are interleaved (per 2-keytile chunk) with
attn@V for head h, the softmax-denominator broadcast matmul for head h is
deferred until the reciprocal has had time to complete, and the output
projection for q-block qb is spread one jt-group per pipeline slot across
q-block qb+1. PSUM budget (8 banks): score chunks 2x[128,2,512] (4) +
attn@V accum 2x[128,512] (2) + denom broadcast (1) + out-proj accum (1).
"""
import sys

sys.path.insert(0, "/opt/trn_rl_repo")

import numpy as np
import ml_dtypes

B, N, E = 4, 2048, 1024
NCORES = 8
HH = 512          # embed dims (8 heads x 64) per core
D = 64
NHEAD = 8         # heads per core

_cache = {}


def _tf32(x):
    u = np.ascontiguousarray(x, dtype=np.float32).view(np.uint32)
    lsb = (u >> 13) & 1
    u = (u + 0x0FFF + lsb) & 0xFFFFE000
    return u.view(np.float32)


def _split_matmul_waits(nc, mybir):
    """fp32r self-loading matmuls cannot carry sync waits (walrus places
    them on the S3_LW struct which has no wait slot). Move every wait off
    fp32r Matmult instructions onto InstEventSemaphore instructions
    inserted just before, in block order.

    Applying this to bf16 matmuls too lets the PE's silicon reorder
    window pre-hoist their (now wait-less) LDWEIGHTS during pipeline
    waits — safe here because every bf16 stationary operand (KT/VE/wo_t)
    is written a full phase before its first consumer — and is worth
    ~120us over native wait-on-LDWEIGHTS placement."""
    n_fixed = 0
    for fn in nc.m.functions:
        for blk in fn.blocks:
            insts = blk.instructions
            i = 0
            while i < len(insts):
                inst = insts[i]
                si = inst.sync_info
                if inst.opcode == "Matmult" and si is not None and len(si.on_wait) > 0:
                    waits = list(si.on_wait)
                    si.on_wait = []
                    inst.sync_info = si
                    pos = i
                    for j in range(0, len(waits), 2):
                        ev = mybir.InstEventSemaphore(
                            name=f"mmgate_{inst.name}_{j}",
                            ins=[],
                            outs=[],
                            sync_info=mybir.SyncInfo(
                                on_wait=waits[j : j + 2], on_update=[]
                            ),
                        )
                        ev.engine = inst.engine
                        nc.register_instruction(ev)
                        insts.insert(pos, ev)
                        pos += 1
                        i += 1
                    n_fixed += 1
                i += 1
            blk.instructions = insts
    return n_fixed


def _build():
    import concourse.mybir as mybir
    import concourse.tile as tile
    import concourse.bacc as bacc

    F32 = mybir.dt.float32
    F32R = mybir.dt.float32r
    BF16 = mybir.dt.bfloat16
    EXP = mybir.ActivationFunctionType.Exp

    nc = bacc.Bacc(trn_type="TRN2")

    xtq = nc.dram_tensor("xtq", [E, N], F32R, kind="ExternalInput")
    xtk = nc.dram_tensor("xtk", [E, N], F32R, kind="ExternalInput")
    xtv = nc.dram_tensor("xtv", [E, N], F32R, kind="ExternalInput")
    wqt = nc.dram_tensor("wqt", [E, HH], F32R, kind="ExternalInput")
    wkt = nc.dram_tensor("wkt", [E, HH], F32R, kind="ExternalInput")
    wvt = nc.dram_tensor("wvt", [E, HH], F32R, kind="ExternalInput")
    wot = nc.dram_tensor("wot", [HH, E], BF16, kind="ExternalInput")
    bq = nc.dram_tensor("bq", [HH], F32, kind="ExternalInput")
    bk = nc.dram_tensor("bk", [HH], F32, kind="ExternalInput")
    bv = nc.dram_tensor("bv", [HH], F32, kind="ExternalInput")
    po = nc.dram_tensor("po", [E, N], F32, kind="ExternalOutput")

    with tile.TileContext(nc) as tc:
        with (
            tc.tile_pool(name="consts", bufs=1) as consts,
            tc.tile_pool(name="qk", bufs=1) as qk_pool,
            tc.tile_pool(name="vx", bufs=1) as v_pool,
            tc.tile_pool(name="wo", bufs=1) as wo_pool,
        ):
            # ---------------- constants ----------------
            ones_f = consts.tile([1, 128], F32)
            nc.vector.memset(ones_f, 1.0)
            ones_r = consts.tile([1, 128], F32R)
            nc.vector.tensor_copy(ones_r, ones_f)
            onescol_f = consts.tile([128, NHEAD, 1], F32)
            nc.vector.memset(onescol_f, 1.0)

            bq_t = consts.tile([128, 4], F32)
            bk_t = consts.tile([128, 4], F32)
            nc.sync.dma_start(out=bq_t, in_=bq.ap().rearrange("(t p) -> p t", p=128))
            nc.sync.dma_start(out=bk_t, in_=bk.ap().rearrange("(t p) -> p t", p=128))
            bv_row = consts.tile([1, HH], F32)
            nc.sync.dma_start(out=bv_row, in_=bv.ap().rearrange("(a n) -> a n", a=1))
            bv_row_r = consts.tile([1, HH], F32R)
            nc.vector.tensor_copy(bv_row_r, bv_row)
            bv_bc = consts.tile([128, HH], F32)

            # persistent activations
            QT = [qk_pool.tile([128, N], BF16, tag=f"qt{t}", name=f"qt{t}") for t in range(4)]
            KT = [qk_pool.tile([128, N], BF16, tag=f"kt{t}", name=f"kt{t}") for t in range(4)]
            VE = [v_pool.tile([128, NHEAD, D + 1], BF16, tag=f"ve{g}", name=f"ve{g}") for g in range(16)]
            wo_t = wo_pool.tile([128, 4, E], BF16, tag="wo")

            # ---------------- projections ----------------
            with (
                tc.tile_pool(name="w", bufs=2) as w_pool,
                tc.tile_pool(name="xt", bufs=2) as xt_pool,
                tc.tile_pool(name="pps", bufs=4, space="PSUM") as proj_ps,
            ):
                # broadcast bv to all partitions via K=1 matmul
                bc0 = proj_ps.tile([128, HH], F32, tag="bvbc")
                nc.tensor.matmul(bc0, ones_r, bv_row_r, start=True, stop=True)
                nc.vector.tensor_copy(bv_bc, bc0)

                w_tiles = {}
                for name, wdram in (("q", wqt), ("k", wkt), ("v", wvt)):
                    wt = w_pool.tile([128, 8, HH], F32R, tag="w")
                    nc.sync.dma_start(
                        out=wt, in_=wdram.ap().rearrange("(kt p) n -> p kt n", p=128)
                    )
                    w_tiles[name] = wt

                def qk_proj(xdram, wt, dest, bias_t):
                    for th in range(4):
                        xt = xt_pool.tile([128, 8, 512], F32R, tag="xt")
                        nc.sync.dma_start(
                            out=xt,
                            in_=xdram.ap().rearrange("(kt p) n -> p kt n", p=128)[
                                :, :, 512 * th : 512 * (th + 1)
                            ],
                        )
                        for dt_ in range(4):
                            ps = proj_ps.tile([128, 512], F32, tag="pp")
                            for kt in range(8):
                                nc.tensor.matmul(
                                    ps,
                                    wt[:, kt, 128 * dt_ : 128 * (dt_ + 1)],
                                    xt[:, kt, :],
                                    start=(kt == 0),
                                    stop=(kt == 7),
                                )
                            off = 512 * th
                            nc.vector.tensor_scalar_add(
                                dest[dt_][:, off : off + 512],
                                ps,
                                bias_t[:, dt_ : dt_ + 1],
                            )

                qk_proj(xtq, w_tiles["q"], QT, bq_t)
                qk_proj(xtk, w_tiles["k"], KT, bk_t)

                # V in natural [tok, dv] layout + ones column
                for th in range(4):
                    xt = xt_pool.tile([128, 8, 512], F32R, tag="xt")
                    nc.sync.dma_start(
                        out=xt,
                        in_=xtv.ap().rearrange("(kt p) n -> p kt n", p=128)[
                            :, :, 512 * th : 512 * (th + 1)
                        ],
                    )
                    for tt in range(4):
                        g = 4 * th + tt
                        ps = proj_ps.tile([128, 512], F32, tag="pp")
                        for kt in range(8):
                            nc.tensor.matmul(
                                ps,
                                xt[:, kt, 128 * tt : 128 * (tt + 1)],
                                w_tiles["v"][:, kt, :],
                                start=(kt == 0),
                                stop=(kt == 7),
                            )
                        nc.vector.tensor_add(
                            VE[g][:, :, 0:D],
                            ps.rearrange("p (h d) -> p h d", h=NHEAD),
                            bv_bc.rearrange("p (h d) -> p h d", h=NHEAD),
                        )
                        nc.vector.tensor_copy(VE[g][:, :, D : D + 1], onescol_f)

                # output projection weights (loaded during attention DMA slack)
                nc.sync.dma_start(
                    out=wo_t, in_=wot.ap().rearrange("(ct p) n -> p ct n", p=128)
                )

            # ---------------- attention (head-pipelined) ----------------
            # Linear head index L = qb*8 + h. Pipeline slot s: issue score
            # chunks for L_S = s and attn@V chunks for L_V = s-1, chunk c
            # covering key tiles 2c, 2c+1. Per chunk the PE does 2 score MMs
            # + 2 attn@V MMs (~850ns warm) while the ACT does one exp over
            # [128,2,512] (~1.05us) — ACT paces, PE never starves.
            with (
                tc.tile_pool(name="attn", bufs=12) as at_pool,
                tc.tile_pool(name="otn", bufs=2) as otn_pool,
                tc.tile_pool(name="small", bufs=2) as small_pool,
                tc.tile_pool(name="ostage", bufs=2) as ostage_pool,
                tc.tile_pool(name="st_ps", bufs=2, space="PSUM") as st_ps,
                tc.tile_pool(name="ot_ps", bufs=2, space="PSUM") as ot_ps,
                tc.tile_pool(name="bc_ps", bufs=1, space="PSUM") as bc_ps,
                tc.tile_pool(name="oj_ps", bufs=1, space="PSUM") as oj_ps,
            ):
                NL = 4 * NHEAD          # 32 (qb, h) pairs
                at_tiles = {}           # (L, c) -> SBUF exp tile
                ot_tiles = {}           # L -> PSUM attn@V accumulator
                rs = {}                 # L -> reciprocal of denominators
                otn_by_qb = {}          # qb -> 4 otn SBUF tiles
                pending_oj = []

                def issue_scores(L, c):
                    qb, h = divmod(L, NHEAD)
                    t, par = h // 2, (h % 2) * 64
                    q0 = 512 * qb
                    st = st_ps.tile([128, 2, 512], F32, tag="st", name=f"st{L}_{c}")
                    for i in range(2):
                        kt = 2 * c + i
                        nc.tensor.matmul(
                            st[:, i, :],
                            KT[t][par : par + 64, 128 * kt : 128 * (kt + 1)],
                            QT[t][par : par + 64, q0 : q0 + 512],
                            start=True,
                            stop=True,
                        )
                    at = at_pool.tile([128, 2, 512], BF16, tag="at", name=f"at{L}_{c}")
                    nc.scalar.activation(at, st, EXP, scale=0.125)
                    at_tiles[(L, c)] = at

                def issue_attnv(L, c):
                    h = L % NHEAD
                    if c == 0:
                        ot_tiles[L] = ot_ps.tile(
                            [128, 512], F32, tag="ot", name=f"ot{L}"
                        )
                    ot = ot_tiles[L]
                    at = at_tiles.pop((L, c))
                    for i in range(2):
                        kt = 2 * c + i
                        nc.tensor.matmul(
                            ot[0:65, :],
                            VE[kt][:, h, :],
                            at[:, i, :],
                            start=(kt == 0),
                            stop=(kt == 15),
                        )

                def issue_recip(L):
                    ot = ot_tiles[L]
                    r = small_pool.tile([1, 512], F32R, tag="r", name=f"r{L}")
                    with nc.allow_low_precision(reason="tf32 softmax denom"):
                        nc.vector.reciprocal(r, ot[64:65, :])
                    rs[L] = r

                def issue_bcmul(L):
                    qb, h = divmod(L, NHEAD)
                    t, par = h // 2, (h % 2) * 64
                    ot = ot_tiles.pop(L)
                    r = rs.pop(L)
                    bc = bc_ps.tile([128, 512], F32, tag="bc", name=f"bc{L}")
                    nc.tensor.matmul(
                        bc[0:64, :], ones_r[:, 0:64], r, start=True, stop=True
                    )
                    rbc = small_pool.tile([64, 512], F32, tag="rbc", name=f"rbc{L}")
                    nc.vector.tensor_copy(rbc, bc[0:64, :])
                    nc.vector.tensor_mul(
                        otn_by_qb[qb][t][par : par + 64, :], ot[0:64, :], rbc
                    )

                def issue_oj(qb, jt):
                    pj = oj_ps.tile([128, 512], F32, tag="oj", name=f"oj{qb}_{jt}")
                    for ct in range(4):
                        nc.tensor.matmul(
                            pj,
                            wo_t[:, ct, 128 * jt : 128 * (jt + 1)],
                            otn_by_qb[qb][ct],
                            start=(ct == 0),
                            stop=(ct == 3),
                        )
                    oj_sb = ostage_pool.tile(
                        [128, 512], F32, tag="oj_sb", name=f"ojsb{qb}_{jt}"
                    )
                    nc.vector.tensor_copy(oj_sb, pj)
                    nc.sync.dma_start(
                        out=po.ap()[128 * jt : 128 * (jt + 1), 512 * qb : 512 * (qb + 1)],
                        in_=oj_sb,
                    )
                    if jt == 7:
                        del otn_by_qb[qb]

                for s in range(NL + 1):
                    L_S = s if s < NL else None
                    L_V = s - 1 if s >= 1 else None
                    if L_S is not None and L_S % NHEAD == 0:
                        qb_S = L_S // NHEAD
                        otn_by_qb[qb_S] = [
                            otn_pool.tile(
                                [128, 512], BF16, tag=f"otn{ct}",
                                name=f"otn{ct}_{qb_S}",
                            )
                            for ct in range(4)
                        ]
                    if L_S is not None and L_S % NHEAD == 2 and L_S // NHEAD >= 1:
                        pending_oj += [(L_S // NHEAD - 1, jt) for jt in range(8)]
                    for c in range(8):
                        if L_V is not None:
                            issue_attnv(L_V, c)
                        if c == 2 and L_V is not None and L_V >= 1:
                            issue_bcmul(L_V - 1)
                        if c == 5 and pending_oj:
                            issue_oj(*pending_oj.pop(0))
                        if L_S is not None:
                            issue_scores(L_S, c)
                    if L_V is not None:
                        issue_recip(L_V)

                # tail: normalization of the last head + remaining out-proj
                issue_bcmul(NL - 1)
                while pending_oj:
                    issue_oj(*pending_oj.pop(0))
                for jt in range(8):
                    issue_oj(3, jt)

    nc.compile()
    _split_matmul_waits(nc, mybir)
    return nc


def _get_nc():
    if "nc" not in _cache:
        _cache["nc"] = _build()
    return _cache["nc"]


def kernel(query, key, value, Wq, bq, Wk, bk, Wv, bv, Wo, bo):
    from concourse.bass_utils import run_bass_kernel_spmd

    nc = _get_nc()

    query = np.asarray(query, dtype=np.float32)
    key = np.asarray(key, dtype=np.float32)
    value = np.asarray(value, dtype=np.float32)
    Wq, Wk, Wv, Wo = (np.asarray(w, dtype=np.float32) for w in (Wq, Wk, Wv, Wo))
    bq, bk, bv, bo = (np.asarray(b, dtype=np.float32) for b in (bq, bk, bv, bo))

    in_maps = []
    for c in range(NCORES):
        b, hh = c // 2, c % 2
        cols = slice(HH * hh, HH * (hh + 1))
        in_maps.append(
            {
                "xtq": _tf32(query[b].T),
                "xtk": _tf32(key[b].T),
                "xtv": _tf32(value[b].T),
                "wqt": _tf32(Wq[cols, :].T),
                "wkt": _tf32(Wk[cols, :].T),
                "wvt": _tf32(Wv[cols, :].T),
                "wot": np.ascontiguousarray(Wo[:, cols].T).astype(ml_dtypes.bfloat16),
                "bq": bq[cols],
                "bk": bk[cols],
                "bv": bv[cols],
            }
        )

    _cache["in_maps"] = in_maps
    res = run_bass_kernel_spmd(nc, in_maps, core_ids=list(range(NCORES)))
    out = np.empty((B, N, E), dtype=np.float32)
    for b in range(B):
        p = res.results[2 * b]["po"] + res.results[2 * b + 1]["po"]
        out[b] = p.T + bo
    return out
